# revision 15
# baseline (speedup 1.0000x reference)
"""Block-wise Hadamard transform kernel for Trainium2 (8 NeuronCores).

y = (x.reshape(-1, 128) @ H).reshape(x.shape)  with x [8192, 8192] f32,
H [128, 128] f32 (symmetric, scaled Hadamard).

Strategy (data-parallel, 1024 rows per core):
  - per 128-row stripe: DMA [128, 8192] into SBUF (rows on partitions)
  - per 512-col group: PE-transpose the four 128x128 blocks into PSUM
    (contraction dim must live on partitions), copy to SBUF, then four
    fp32 matmuls lhsT=x_blk^T, rhs=H -> y natural layout in PSUM,
    copy to the output stripe, DMA stripe out.
"""

import sys

sys.path.insert(0, "/opt/trn_rl_repo")

import numpy as np

import concourse.bass as bass
import concourse.mybir as mybir
from concourse.masks import make_identity
from concourse.tile import TileContext

N_CORES = 8
ROWS, COLS = 8192, 8192
SHARD = ROWS // N_CORES  # 1024 rows per core
P = 128
GROUP = 512  # one PSUM bank of fp32; 4 blocks of 128
F32 = mybir.dt.float32

_cached_nc = None


def _legalize_waits(nc, max_waits=1):
    """Walrus in this container accepts only one sync-wait slot per engine
    instruction. Move surplus waits onto injected same-engine NoOps (engines
    execute their stream in order, so semantics are preserved)."""
    n = 0
    for f in nc.m.functions:
        for bb in f.blocks:
            new = []
            for inst in bb.instructions:
                si = inst.sync_info
                waits = list(si.on_wait) if si is not None else []
                if len(waits) > max_waits:
                    keep = waits[-max_waits:]
                    for w in waits[:-max_waits]:
                        nop = mybir.InstNoOp(name=f"legw-{n}")
                        n += 1
                        nop.engine = inst.engine
                        nop.sync_info = mybir.SyncInfo(on_wait=[w], on_update=[])
                        new.append(nop)
                    inst.sync_info = mybir.SyncInfo(
                        on_wait=keep, on_update=list(si.on_update)
                    )
                new.append(inst)
            bb.instructions = new
    return nc


def _build():
    nc = bass.Bass()
    x = nc.declare_dram_parameter("x", [SHARD, COLS], F32, isOutput=False)
    h = nc.declare_dram_parameter("H", [P, P], F32, isOutput=False)
    y = nc.declare_dram_parameter("y", [SHARD, COLS], F32, isOutput=True)

    n_stripes = SHARD // P  # 8
    n_groups = COLS // GROUP  # 16

    with TileContext(nc) as tc:
        with (
            tc.tile_pool(name="consts", bufs=1) as consts,
            tc.tile_pool(name="xs", bufs=3) as xpool,
            tc.tile_pool(name="ys", bufs=2) as ypool,
            tc.tile_pool(name="xt", bufs=4) as xtpool,
            tc.tile_pool(name="pst", bufs=3, space="PSUM") as pst,
            tc.tile_pool(name="psy", bufs=4, space="PSUM") as psy,
            tc.tile_pool(name="warm", bufs=1, space="PSUM") as warm,
        ):
            h_tile = consts.tile([P, P], F32)
            nc.sync.dma_start(out=h_tile, in_=h[:, :])
            ident = consts.tile([P, P], F32)
            make_identity(nc, ident)
            # prime PE on h_tile and ident so later matmuls/transposes don't
            # carry those semaphore waits (walrus caps sync waits per Matmult;
            # the LDWEIGHTS slot takes only one)
            warm_ps = warm.tile([P, P], F32, tag="w")
            nc.tensor.matmul(warm_ps, h_tile, h_tile, start=True, stop=True)
            warm_ps2 = warm.tile([P, P], F32, tag="w")
            nc.tensor.transpose(warm_ps2, h_tile, ident)
            # keep the PE HAM-warm through the first stripe load (~10.5 us):
            # otherwise it re-throttles to 1.2 GHz and stripe 0 runs cold
            for _ in range(40):
                wp = warm.tile([P, P], F32, tag="w")
                nc.tensor.matmul(wp, h_tile, h_tile, start=True, stop=True)

            for s in range(n_stripes):
                xs = xpool.tile([P, COLS], F32)
                # loads ride the SP HWDGE ring; stores ride the ACT ring, so
                # a queued store never blocks the next stripe's load (HWDGE
                # is FIFO per issuing engine)
                nc.sync.dma_start(out=xs, in_=x[s * P : (s + 1) * P, :])
                ys = ypool.tile([P, COLS], F32)
                for g in range(n_groups):
                    g0 = g * GROUP
                    xt_ps = pst.tile([P, GROUP], F32)
                    for b in range(GROUP // P):
                        nc.tensor.transpose(
                            xt_ps[:, b * P : (b + 1) * P],
                            xs[:, g0 + b * P : g0 + (b + 1) * P],
                            ident,
                        )
                    xt = xtpool.tile([P, GROUP], F32)
                    nc.vector.tensor_copy(xt, xt_ps)
                    y_ps = psy.tile([P, GROUP], F32)
                    for b in range(GROUP // P):
                        nc.tensor.matmul(
                            y_ps[:, b * P : (b + 1) * P],
                            xt[:, b * P : (b + 1) * P],
                            h_tile,
                            start=True,
                            stop=True,
                        )
                    # y drains go to the otherwise-idle scalar engine; the
                    # wait-legalization pass absorbs the extra sem waits
                    nc.scalar.copy(ys[:, g0 : g0 + GROUP], y_ps)
                nc.scalar.dma_start(out=y[s * P : (s + 1) * P, :], in_=ys)
    return _legalize_waits(nc)


def kernel(x, H, trace=False):
    global _cached_nc
    if _cached_nc is None:
        _cached_nc = _build()
    nc = _cached_nc

    from concourse.bass_utils import run_bass_kernel_spmd

    if trace:
        # register the axon NTFF hook (the agent image lacks the boot-time
        # registration; inject the registry module and replicate the hook
        # from trn_agent_boot)
        import types

        if "antenv.axon_hooks" not in sys.modules:
            mod = types.ModuleType("antenv.axon_hooks")
            mod._hook = None
            mod.set_axon_ntff_profile_hook = lambda h: setattr(mod, "_hook", h)
            mod.get_axon_ntff_profile_hook = lambda: mod._hook
            sys.modules["antenv.axon_hooks"] = mod
            import antenv

            antenv.axon_hooks = mod
        hooks = sys.modules["antenv.axon_hooks"]
        if hooks.get_axon_ntff_profile_hook() is None:
            if "/root/.axon_site" not in sys.path:
                sys.path.insert(0, "/root/.axon_site")
            from trn_agent_boot.trn_boot import _ntff_profile_via_ctypes

            hooks.set_axon_ntff_profile_hook(
                _ntff_profile_via_ctypes("/opt/axon/libaxon_pjrt.so")
            )

    x = np.asarray(x)
    H = np.asarray(H)
    in_maps = [
        {"x": x[i * SHARD : (i + 1) * SHARD], "H": H} for i in range(N_CORES)
    ]
    res = run_bass_kernel_spmd(
        nc, in_maps, core_ids=list(range(N_CORES)), trace=trace
    )
    out = np.concatenate([r["y"] for r in res.results], axis=0)
    if trace:
        kernel.last_exec_time_ns = res.exec_time_ns
        kernel.last_trace = res.instructions_and_trace
    return out


# revision 16
# speedup vs baseline: 1.1121x; 1.1121x over previous
"""Block-wise Hadamard transform kernel for Trainium2 (8 NeuronCores).

y = (x.reshape(-1, 128) @ H).reshape(x.shape)  with x [8192, 8192] f32,
H [128, 128] f32 (symmetric, scaled Hadamard).

Strategy (data-parallel, 1024 rows per core):
  - per 128-row stripe: DMA [128, 8192] into SBUF (rows on partitions)
  - per 512-col group: PE-transpose the four 128x128 blocks into PSUM
    (contraction dim must live on partitions), copy to SBUF, then four
    fp32 matmuls lhsT=x_blk^T, rhs=H -> y natural layout in PSUM,
    copy to the output stripe, DMA stripe out.
"""

import sys

sys.path.insert(0, "/opt/trn_rl_repo")

import numpy as np

import concourse.bass as bass
import concourse.mybir as mybir
from concourse.masks import make_identity
from concourse.tile import TileContext

N_CORES = 8
ROWS, COLS = 8192, 8192
SHARD = ROWS // N_CORES  # 1024 rows per core
P = 128
GROUP = 512  # one PSUM bank of fp32; 4 blocks of 128
F32 = mybir.dt.float32

_cached_nc = None


def _legalize_waits(nc, max_waits=1):
    """Walrus in this container accepts only one sync-wait slot per engine
    instruction. Move surplus waits onto injected same-engine NoOps (engines
    execute their stream in order, so semantics are preserved)."""
    n = 0
    for f in nc.m.functions:
        for bb in f.blocks:
            new = []
            for inst in bb.instructions:
                si = inst.sync_info
                waits = list(si.on_wait) if si is not None else []
                if len(waits) > max_waits:
                    keep = waits[-max_waits:]
                    for w in waits[:-max_waits]:
                        nop = mybir.InstNoOp(name=f"legw-{n}")
                        n += 1
                        nop.engine = inst.engine
                        nop.sync_info = mybir.SyncInfo(on_wait=[w], on_update=[])
                        new.append(nop)
                    inst.sync_info = mybir.SyncInfo(
                        on_wait=keep, on_update=list(si.on_update)
                    )
                new.append(inst)
            bb.instructions = new
    return nc


def _build():
    nc = bass.Bass()
    x = nc.declare_dram_parameter("x", [SHARD, COLS], F32, isOutput=False)
    h = nc.declare_dram_parameter("H", [P, P], F32, isOutput=False)
    y = nc.declare_dram_parameter("y", [SHARD, COLS], F32, isOutput=True)

    n_stripes = SHARD // P  # 8
    n_groups = COLS // GROUP  # 16

    with TileContext(nc) as tc:
        with (
            tc.tile_pool(name="consts", bufs=1) as consts,
            tc.tile_pool(name="xs", bufs=3) as xpool,
            tc.tile_pool(name="ys", bufs=2) as ypool,
            tc.tile_pool(name="xt", bufs=4) as xtpool,
            tc.tile_pool(name="pst", bufs=3, space="PSUM") as pst,
            tc.tile_pool(name="psy", bufs=4, space="PSUM") as psy,
            tc.tile_pool(name="warm", bufs=1, space="PSUM") as warm,
        ):
            h_tile = consts.tile([P, P], F32)
            nc.sync.dma_start(out=h_tile, in_=h[:, :])
            ident = consts.tile([P, P], F32)
            make_identity(nc, ident)
            # prime PE on h_tile and ident so later matmuls/transposes don't
            # carry those semaphore waits (walrus caps sync waits per Matmult;
            # the LDWEIGHTS slot takes only one)
            warm_ps = warm.tile([P, P], F32, tag="w")
            nc.tensor.matmul(warm_ps, h_tile, h_tile, start=True, stop=True)
            warm_ps2 = warm.tile([P, P], F32, tag="w")
            nc.tensor.transpose(warm_ps2, h_tile, ident)
            # keep the PE HAM-warm through the first stripe load (~10.5 us):
            # otherwise it re-throttles to 1.2 GHz and stripe 0 runs cold.
            # Reuse one tile: same-engine WAW needs no semaphores.
            for _ in range(40):
                nc.tensor.matmul(warm_ps2, h_tile, h_tile, start=True, stop=True)

            for s in range(n_stripes):
                xs = xpool.tile([P, COLS], F32)
                # loads ride the SP HWDGE ring; stores ride the ACT ring, so
                # a queued store never blocks the next stripe's load (HWDGE
                # is FIFO per issuing engine)
                nc.sync.dma_start(out=xs, in_=x[s * P : (s + 1) * P, :])
                ys = ypool.tile([P, COLS], F32)
                for g in range(n_groups):
                    g0 = g * GROUP
                    xt_ps = pst.tile([P, GROUP], F32)
                    for b in range(GROUP // P):
                        nc.tensor.transpose(
                            xt_ps[:, b * P : (b + 1) * P],
                            xs[:, g0 + b * P : g0 + (b + 1) * P],
                            ident,
                        )
                    xt = xtpool.tile([P, GROUP], F32)
                    nc.vector.tensor_copy(xt, xt_ps)
                    y_ps = psy.tile([P, GROUP], F32)
                    for b in range(GROUP // P):
                        nc.tensor.matmul(
                            y_ps[:, b * P : (b + 1) * P],
                            xt[:, b * P : (b + 1) * P],
                            h_tile,
                            start=True,
                            stop=True,
                        )
                    # y drains go to the otherwise-idle scalar engine; the
                    # wait-legalization pass absorbs the extra sem waits
                    nc.scalar.copy(ys[:, g0 : g0 + GROUP], y_ps)
                nc.scalar.dma_start(out=y[s * P : (s + 1) * P, :], in_=ys)
    return _legalize_waits(nc)


def kernel(x, H, trace=False):
    global _cached_nc
    if _cached_nc is None:
        _cached_nc = _build()
    nc = _cached_nc

    from concourse.bass_utils import run_bass_kernel_spmd

    if trace:
        # register the axon NTFF hook (the agent image lacks the boot-time
        # registration; inject the registry module and replicate the hook
        # from trn_agent_boot)
        import types

        if "antenv.axon_hooks" not in sys.modules:
            mod = types.ModuleType("antenv.axon_hooks")
            mod._hook = None
            mod.set_axon_ntff_profile_hook = lambda h: setattr(mod, "_hook", h)
            mod.get_axon_ntff_profile_hook = lambda: mod._hook
            sys.modules["antenv.axon_hooks"] = mod
            import antenv

            antenv.axon_hooks = mod
        hooks = sys.modules["antenv.axon_hooks"]
        if hooks.get_axon_ntff_profile_hook() is None:
            if "/root/.axon_site" not in sys.path:
                sys.path.insert(0, "/root/.axon_site")
            from trn_agent_boot.trn_boot import _ntff_profile_via_ctypes

            hooks.set_axon_ntff_profile_hook(
                _ntff_profile_via_ctypes("/opt/axon/libaxon_pjrt.so")
            )

    x = np.asarray(x)
    H = np.asarray(H)
    in_maps = [
        {"x": x[i * SHARD : (i + 1) * SHARD], "H": H} for i in range(N_CORES)
    ]
    res = run_bass_kernel_spmd(
        nc, in_maps, core_ids=list(range(N_CORES)), trace=trace
    )
    out = np.concatenate([r["y"] for r in res.results], axis=0)
    if trace:
        kernel.last_exec_time_ns = res.exec_time_ns
        kernel.last_trace = res.instructions_and_trace
    return out


# revision 19
# speedup vs baseline: 1.1552x; 1.0388x over previous
"""Block-wise Hadamard transform kernel for Trainium2 (8 NeuronCores).

y = (x.reshape(-1, 128) @ H).reshape(x.shape)  with x [8192, 8192] f32,
H [128, 128] f32 (symmetric, scaled Hadamard).

Strategy (data-parallel, 1024 rows per core):
  - per 128-row stripe: DMA [128, 8192] into SBUF (rows on partitions)
  - per 512-col group: PE-transpose the four 128x128 blocks into PSUM
    (contraction dim must live on partitions), copy to SBUF, then four
    fp32 matmuls lhsT=x_blk^T, rhs=H -> y natural layout in PSUM,
    copy to the output stripe, DMA stripe out.
"""

import sys

sys.path.insert(0, "/opt/trn_rl_repo")

import numpy as np

import concourse.bass as bass
import concourse.mybir as mybir
from concourse.masks import make_identity
from concourse.tile import TileContext

N_CORES = 8
ROWS, COLS = 8192, 8192
SHARD = ROWS // N_CORES  # 1024 rows per core
P = 128
GROUP = 512  # one PSUM bank of fp32; 4 blocks of 128
F32 = mybir.dt.float32

_cached_nc = None


def _legalize_waits(nc, max_waits=1):
    """Walrus in this container accepts only one sync-wait slot per engine
    instruction. Move surplus waits onto injected same-engine NoOps (engines
    execute their stream in order, so semantics are preserved)."""
    n = 0
    for f in nc.m.functions:
        for bb in f.blocks:
            new = []
            for inst in bb.instructions:
                si = inst.sync_info
                waits = list(si.on_wait) if si is not None else []
                if len(waits) > max_waits:
                    keep = waits[-max_waits:]
                    for w in waits[:-max_waits]:
                        nop = mybir.InstNoOp(name=f"legw-{n}")
                        n += 1
                        nop.engine = inst.engine
                        nop.sync_info = mybir.SyncInfo(on_wait=[w], on_update=[])
                        new.append(nop)
                    inst.sync_info = mybir.SyncInfo(
                        on_wait=keep, on_update=list(si.on_update)
                    )
                new.append(inst)
            bb.instructions = new
    return nc


def _build():
    nc = bass.Bass()
    x = nc.declare_dram_parameter("x", [SHARD, COLS], F32, isOutput=False)
    h = nc.declare_dram_parameter("H", [P, P], F32, isOutput=False)
    y = nc.declare_dram_parameter("y", [SHARD, COLS], F32, isOutput=True)

    n_stripes = SHARD // P  # 8
    n_groups = COLS // GROUP  # 16

    with TileContext(nc) as tc:
        with (
            tc.tile_pool(name="consts", bufs=1) as consts,
            tc.tile_pool(name="xs", bufs=3) as xpool,
            tc.tile_pool(name="ys", bufs=2) as ypool,
            tc.tile_pool(name="xt", bufs=4) as xtpool,
            tc.tile_pool(name="pst", bufs=3, space="PSUM") as pst,
            tc.tile_pool(name="psy", bufs=4, space="PSUM") as psy,
            tc.tile_pool(name="warm", bufs=1, space="PSUM") as warm,
        ):
            h_tile = consts.tile([P, P], F32)
            nc.sync.dma_start(out=h_tile, in_=h[:, :])
            ident = consts.tile([P, P], F32)
            make_identity(nc, ident)
            # prime PE on h_tile and ident so later matmuls/transposes don't
            # carry those semaphore waits (walrus caps sync waits per Matmult;
            # the LDWEIGHTS slot takes only one)
            warm_ps = warm.tile([P, P], F32, tag="w")
            nc.tensor.matmul(warm_ps, h_tile, h_tile, start=True, stop=True)
            warm_ps2 = warm.tile([P, P], F32, tag="w")
            nc.tensor.transpose(warm_ps2, h_tile, ident)
            # keep the PE HAM-warm until the first load chunk lands (~3 us):
            # otherwise it re-throttles to 1.2 GHz and stripe 0 runs cold.
            # Reuse one tile: same-engine WAW needs no semaphores.
            for _ in range(12):
                nc.tensor.matmul(warm_ps2, h_tile, h_tile, start=True, stop=True)

            for s in range(n_stripes):
                xs = xpool.tile([P, COLS], F32)
                # loads ride the SP HWDGE ring; stores ride the ACT ring, so
                # a queued store never blocks the next stripe's load (HWDGE
                # is FIFO per issuing engine). First stripe loads in 1 MiB
                # chunks so compute starts ~2.6 us in instead of ~10.5 us.
                if s == 0:
                    q = COLS // 4
                    for c in range(4):
                        nc.sync.dma_start(
                            out=xs[:, c * q : (c + 1) * q],
                            in_=x[s * P : (s + 1) * P, c * q : (c + 1) * q],
                        )
                else:
                    nc.sync.dma_start(out=xs, in_=x[s * P : (s + 1) * P, :])
                ys = ypool.tile([P, COLS], F32)
                for g in range(n_groups):
                    g0 = g * GROUP
                    xt_ps = pst.tile([P, GROUP], F32)
                    for b in range(GROUP // P):
                        nc.tensor.transpose(
                            xt_ps[:, b * P : (b + 1) * P],
                            xs[:, g0 + b * P : g0 + (b + 1) * P],
                            ident,
                        )
                    xt = xtpool.tile([P, GROUP], F32)
                    nc.vector.tensor_copy(xt, xt_ps)
                    y_ps = psy.tile([P, GROUP], F32)
                    for b in range(GROUP // P):
                        nc.tensor.matmul(
                            y_ps[:, b * P : (b + 1) * P],
                            xt[:, b * P : (b + 1) * P],
                            h_tile,
                            start=True,
                            stop=True,
                        )
                    # y drains go to the otherwise-idle scalar engine; the
                    # wait-legalization pass absorbs the extra sem waits
                    nc.scalar.copy(ys[:, g0 : g0 + GROUP], y_ps)
                # last stripe streams out in 1 MiB chunks so the kernel tail
                # is one chunk, not a whole 4 MiB stripe store
                if s == n_stripes - 1:
                    q = COLS // 4
                    for c in range(4):
                        nc.scalar.dma_start(
                            out=y[s * P : (s + 1) * P, c * q : (c + 1) * q],
                            in_=ys[:, c * q : (c + 1) * q],
                        )
                else:
                    nc.scalar.dma_start(out=y[s * P : (s + 1) * P, :], in_=ys)
    return _legalize_waits(nc)


def kernel(x, H, trace=False):
    global _cached_nc
    if _cached_nc is None:
        _cached_nc = _build()
    nc = _cached_nc

    from concourse.bass_utils import run_bass_kernel_spmd

    if trace:
        # register the axon NTFF hook (the agent image lacks the boot-time
        # registration; inject the registry module and replicate the hook
        # from trn_agent_boot)
        import types

        if "antenv.axon_hooks" not in sys.modules:
            mod = types.ModuleType("antenv.axon_hooks")
            mod._hook = None
            mod.set_axon_ntff_profile_hook = lambda h: setattr(mod, "_hook", h)
            mod.get_axon_ntff_profile_hook = lambda: mod._hook
            sys.modules["antenv.axon_hooks"] = mod
            import antenv

            antenv.axon_hooks = mod
        hooks = sys.modules["antenv.axon_hooks"]
        if hooks.get_axon_ntff_profile_hook() is None:
            if "/root/.axon_site" not in sys.path:
                sys.path.insert(0, "/root/.axon_site")
            from trn_agent_boot.trn_boot import _ntff_profile_via_ctypes

            hooks.set_axon_ntff_profile_hook(
                _ntff_profile_via_ctypes("/opt/axon/libaxon_pjrt.so")
            )

    x = np.asarray(x)
    H = np.asarray(H)
    in_maps = [
        {"x": x[i * SHARD : (i + 1) * SHARD], "H": H} for i in range(N_CORES)
    ]
    res = run_bass_kernel_spmd(
        nc, in_maps, core_ids=list(range(N_CORES)), trace=trace
    )
    out = np.concatenate([r["y"] for r in res.results], axis=0)
    if trace:
        kernel.last_exec_time_ns = res.exec_time_ns
        kernel.last_trace = res.instructions_and_trace
    return out


# revision 20
# speedup vs baseline: 1.1882x; 1.0285x over previous
"""Block-wise Hadamard transform kernel for Trainium2 (8 NeuronCores).

y = (x.reshape(-1, 128) @ H).reshape(x.shape)  with x [8192, 8192] f32,
H [128, 128] f32 (symmetric, scaled Hadamard).

Strategy (data-parallel, 1024 rows per core):
  - per 128-row stripe: DMA [128, 8192] into SBUF (rows on partitions)
  - per 512-col group: PE-transpose the four 128x128 blocks into PSUM
    (contraction dim must live on partitions), copy to SBUF, then four
    fp32 matmuls lhsT=x_blk^T, rhs=H -> y natural layout in PSUM,
    copy to the output stripe, DMA stripe out.
"""

import sys

sys.path.insert(0, "/opt/trn_rl_repo")

import numpy as np

import concourse.bass as bass
import concourse.mybir as mybir
from concourse.masks import make_identity
from concourse.tile import TileContext

N_CORES = 8
ROWS, COLS = 8192, 8192
SHARD = ROWS // N_CORES  # 1024 rows per core
P = 128
GROUP = 512  # one PSUM bank of fp32; 4 blocks of 128
F32 = mybir.dt.float32

_cached_nc = None


def _legalize_waits(nc, max_waits=1):
    """Walrus in this container accepts only one sync-wait slot per engine
    instruction. Move surplus waits onto injected same-engine NoOps (engines
    execute their stream in order, so semantics are preserved)."""
    n = 0
    for f in nc.m.functions:
        for bb in f.blocks:
            new = []
            for inst in bb.instructions:
                si = inst.sync_info
                waits = list(si.on_wait) if si is not None else []
                if len(waits) > max_waits:
                    keep = waits[-max_waits:]
                    for w in waits[:-max_waits]:
                        nop = mybir.InstNoOp(name=f"legw-{n}")
                        n += 1
                        nop.engine = inst.engine
                        nop.sync_info = mybir.SyncInfo(on_wait=[w], on_update=[])
                        new.append(nop)
                    inst.sync_info = mybir.SyncInfo(
                        on_wait=keep, on_update=list(si.on_update)
                    )
                new.append(inst)
            bb.instructions = new
    return nc


def _build():
    nc = bass.Bass()
    x = nc.declare_dram_parameter("x", [SHARD, COLS], F32, isOutput=False)
    h = nc.declare_dram_parameter("H", [P, P], F32, isOutput=False)
    y = nc.declare_dram_parameter("y", [SHARD, COLS], F32, isOutput=True)

    n_stripes = SHARD // P  # 8
    n_groups = COLS // GROUP  # 16

    with TileContext(nc) as tc:
        with (
            tc.tile_pool(name="consts", bufs=1) as consts,
            tc.tile_pool(name="xs", bufs=3) as xpool,
            tc.tile_pool(name="ys", bufs=2) as ypool,
            tc.tile_pool(name="xt", bufs=4) as xtpool,
            tc.tile_pool(name="pst", bufs=3, space="PSUM") as pst,
            tc.tile_pool(name="psy", bufs=4, space="PSUM") as psy,
            tc.tile_pool(name="warm", bufs=1, space="PSUM") as warm,
        ):
            h_tile = consts.tile([P, P], F32)
            nc.sync.dma_start(out=h_tile, in_=h[:, :])
            ident = consts.tile([P, P], F32)
            make_identity(nc, ident)
            # prime PE on h_tile and ident so later matmuls/transposes don't
            # carry those semaphore waits (walrus caps sync waits per Matmult;
            # the LDWEIGHTS slot takes only one)
            warm_ps = warm.tile([P, P], F32, tag="w")
            nc.tensor.matmul(warm_ps, h_tile, h_tile, start=True, stop=True)
            warm_ps2 = warm.tile([P, P], F32, tag="w")
            nc.tensor.transpose(warm_ps2, h_tile, ident)
            # keep the PE HAM-warm until the first load chunk lands (~3 us):
            # otherwise it re-throttles to 1.2 GHz and stripe 0 runs cold.
            # Reuse one tile: same-engine WAW needs no semaphores.
            for _ in range(12):
                nc.tensor.matmul(warm_ps2, h_tile, h_tile, start=True, stop=True)

            for s in range(n_stripes):
                xs = xpool.tile([P, COLS], F32)
                # loads ride the SP HWDGE ring; stores ride the ACT ring, so
                # a queued store never blocks the next stripe's load (HWDGE
                # is FIFO per issuing engine). First stripe loads in 1 MiB
                # chunks so compute starts ~2.6 us in instead of ~10.5 us.
                if s == 0:
                    q = COLS // 4
                    for c in range(4):
                        nc.sync.dma_start(
                            out=xs[:, c * q : (c + 1) * q],
                            in_=x[s * P : (s + 1) * P, c * q : (c + 1) * q],
                        )
                else:
                    nc.sync.dma_start(out=xs, in_=x[s * P : (s + 1) * P, :])
                ys = ypool.tile([P, COLS], F32)
                # software-pipeline by one group: emit group g's transposes
                # before group g-1's matmuls so the PE has independent work
                # while the DVE drains group g-1's transposed blocks
                pending = None

                def flush_pending():
                    nonlocal pending
                    if pending is None:
                        return
                    xt_p, g0_p = pending
                    y_ps = psy.tile([P, GROUP], F32)
                    for b in range(GROUP // P):
                        nc.tensor.matmul(
                            y_ps[:, b * P : (b + 1) * P],
                            xt_p[:, b * P : (b + 1) * P],
                            h_tile,
                            start=True,
                            stop=True,
                        )
                    # y drains go to the otherwise-idle scalar engine; the
                    # wait-legalization pass absorbs the extra sem waits
                    nc.scalar.copy(ys[:, g0_p : g0_p + GROUP], y_ps)
                    pending = None

                for g in range(n_groups):
                    g0 = g * GROUP
                    xt_ps = pst.tile([P, GROUP], F32)
                    for b in range(GROUP // P):
                        nc.tensor.transpose(
                            xt_ps[:, b * P : (b + 1) * P],
                            xs[:, g0 + b * P : g0 + (b + 1) * P],
                            ident,
                        )
                    xt = xtpool.tile([P, GROUP], F32)
                    nc.vector.tensor_copy(xt, xt_ps)
                    flush_pending()
                    pending = (xt, g0)
                flush_pending()
                # last stripe streams out in 1 MiB chunks so the kernel tail
                # is one chunk, not a whole 4 MiB stripe store
                if s == n_stripes - 1:
                    q = COLS // 4
                    for c in range(4):
                        nc.scalar.dma_start(
                            out=y[s * P : (s + 1) * P, c * q : (c + 1) * q],
                            in_=ys[:, c * q : (c + 1) * q],
                        )
                else:
                    nc.scalar.dma_start(out=y[s * P : (s + 1) * P, :], in_=ys)
    return _legalize_waits(nc)


def kernel(x, H, trace=False):
    global _cached_nc
    if _cached_nc is None:
        _cached_nc = _build()
    nc = _cached_nc

    from concourse.bass_utils import run_bass_kernel_spmd

    if trace:
        # register the axon NTFF hook (the agent image lacks the boot-time
        # registration; inject the registry module and replicate the hook
        # from trn_agent_boot)
        import types

        if "antenv.axon_hooks" not in sys.modules:
            mod = types.ModuleType("antenv.axon_hooks")
            mod._hook = None
            mod.set_axon_ntff_profile_hook = lambda h: setattr(mod, "_hook", h)
            mod.get_axon_ntff_profile_hook = lambda: mod._hook
            sys.modules["antenv.axon_hooks"] = mod
            import antenv

            antenv.axon_hooks = mod
        hooks = sys.modules["antenv.axon_hooks"]
        if hooks.get_axon_ntff_profile_hook() is None:
            if "/root/.axon_site" not in sys.path:
                sys.path.insert(0, "/root/.axon_site")
            from trn_agent_boot.trn_boot import _ntff_profile_via_ctypes

            hooks.set_axon_ntff_profile_hook(
                _ntff_profile_via_ctypes("/opt/axon/libaxon_pjrt.so")
            )

    x = np.asarray(x)
    H = np.asarray(H)
    in_maps = [
        {"x": x[i * SHARD : (i + 1) * SHARD], "H": H} for i in range(N_CORES)
    ]
    res = run_bass_kernel_spmd(
        nc, in_maps, core_ids=list(range(N_CORES)), trace=trace
    )
    out = np.concatenate([r["y"] for r in res.results], axis=0)
    if trace:
        kernel.last_exec_time_ns = res.exec_time_ns
        kernel.last_trace = res.instructions_and_trace
    return out


# revision 21
# speedup vs baseline: 1.2039x; 1.0133x over previous
"""Block-wise Hadamard transform kernel for Trainium2 (8 NeuronCores).

y = (x.reshape(-1, 128) @ H).reshape(x.shape)  with x [8192, 8192] f32,
H [128, 128] f32 (symmetric, scaled Hadamard).

Strategy (data-parallel, 1024 rows per core):
  - per 128-row stripe: DMA [128, 8192] into SBUF (rows on partitions)
  - per 512-col group: PE-transpose the four 128x128 blocks into PSUM
    (contraction dim must live on partitions), copy to SBUF, then four
    fp32 matmuls lhsT=x_blk^T, rhs=H -> y natural layout in PSUM,
    copy to the output stripe, DMA stripe out.
"""

import sys

sys.path.insert(0, "/opt/trn_rl_repo")

import numpy as np

import concourse.bass as bass
import concourse.mybir as mybir
from concourse.masks import make_identity
from concourse.tile import TileContext

N_CORES = 8
ROWS, COLS = 8192, 8192
SHARD = ROWS // N_CORES  # 1024 rows per core
P = 128
GROUP = 512  # one PSUM bank of fp32; 4 blocks of 128
F32 = mybir.dt.float32

_cached_nc = None


def _legalize_waits(nc, max_waits=1):
    """Walrus in this container accepts only one sync-wait slot per engine
    instruction. Move surplus waits onto injected same-engine NoOps (engines
    execute their stream in order, so semantics are preserved)."""
    n = 0
    for f in nc.m.functions:
        for bb in f.blocks:
            new = []
            for inst in bb.instructions:
                si = inst.sync_info
                waits = list(si.on_wait) if si is not None else []
                if len(waits) > max_waits:
                    keep = waits[-max_waits:]
                    for w in waits[:-max_waits]:
                        nop = mybir.InstNoOp(name=f"legw-{n}")
                        n += 1
                        nop.engine = inst.engine
                        nop.sync_info = mybir.SyncInfo(on_wait=[w], on_update=[])
                        new.append(nop)
                    inst.sync_info = mybir.SyncInfo(
                        on_wait=keep, on_update=list(si.on_update)
                    )
                new.append(inst)
            bb.instructions = new
    return nc


def _build():
    nc = bass.Bass()
    x = nc.declare_dram_parameter("x", [SHARD, COLS], F32, isOutput=False)
    h = nc.declare_dram_parameter("H", [P, P], F32, isOutput=False)
    y = nc.declare_dram_parameter("y", [SHARD, COLS], F32, isOutput=True)

    n_stripes = SHARD // P  # 8
    n_groups = COLS // GROUP  # 16

    with TileContext(nc) as tc:
        with (
            tc.tile_pool(name="consts", bufs=1) as consts,
            tc.tile_pool(name="xs", bufs=3) as xpool,
            tc.tile_pool(name="ys", bufs=2) as ypool,
            tc.tile_pool(name="xt", bufs=4) as xtpool,
            tc.tile_pool(name="pst", bufs=3, space="PSUM") as pst,
            tc.tile_pool(name="psy", bufs=4, space="PSUM") as psy,
            tc.tile_pool(name="warm", bufs=1, space="PSUM") as warm,
        ):
            h_tile = consts.tile([P, P], F32)
            nc.sync.dma_start(out=h_tile, in_=h[:, :])
            ident = consts.tile([P, P], F32)
            make_identity(nc, ident)
            # prime PE on h_tile and ident so later matmuls/transposes don't
            # carry those semaphore waits (walrus caps sync waits per Matmult;
            # the LDWEIGHTS slot takes only one)
            warm_ps = warm.tile([P, P], F32, tag="w")
            nc.tensor.matmul(warm_ps, h_tile, h_tile, start=True, stop=True)
            warm_ps2 = warm.tile([P, P], F32, tag="w")
            nc.tensor.transpose(warm_ps2, h_tile, ident)
            # keep the PE HAM-warm until the first load chunk lands (~3 us):
            # otherwise it re-throttles to 1.2 GHz and stripe 0 runs cold.
            # Reuse one tile: same-engine WAW needs no semaphores.
            for _ in range(12):
                nc.tensor.matmul(warm_ps2, h_tile, h_tile, start=True, stop=True)

            for s in range(n_stripes):
                xs = xpool.tile([P, COLS], F32)
                # loads ride the SP HWDGE ring; stores ride the ACT ring, so
                # a queued store never blocks the next stripe's load (HWDGE
                # is FIFO per issuing engine). First stripe loads in 1 MiB
                # chunks so compute starts ~2.6 us in instead of ~10.5 us.
                if s == 0:
                    q = COLS // 4
                    for c in range(4):
                        nc.sync.dma_start(
                            out=xs[:, c * q : (c + 1) * q],
                            in_=x[s * P : (s + 1) * P, c * q : (c + 1) * q],
                        )
                else:
                    nc.sync.dma_start(out=xs, in_=x[s * P : (s + 1) * P, :])
                ys = ypool.tile([P, COLS], F32)
                # software-pipeline by one group: emit group g's transposes
                # before group g-1's matmuls so the PE has independent work
                # while the DVE drains group g-1's transposed blocks
                pending = None

                def flush_pending():
                    nonlocal pending
                    if pending is None:
                        return
                    xt_p, g0_p = pending
                    y_ps = psy.tile([P, GROUP], F32)
                    for b in range(GROUP // P):
                        nc.tensor.matmul(
                            y_ps[:, b * P : (b + 1) * P],
                            xt_p[:, b * P : (b + 1) * P],
                            h_tile,
                            start=True,
                            stop=True,
                        )
                    # y drains go to the otherwise-idle scalar engine; the
                    # wait-legalization pass absorbs the extra sem waits
                    nc.scalar.copy(ys[:, g0_p : g0_p + GROUP], y_ps)
                    pending = None

                for g in range(n_groups):
                    g0 = g * GROUP
                    xt_ps = pst.tile([P, GROUP], F32)
                    for b in range(GROUP // P):
                        nc.tensor.transpose(
                            xt_ps[:, b * P : (b + 1) * P],
                            xs[:, g0 + b * P : g0 + (b + 1) * P],
                            ident,
                        )
                    xt = xtpool.tile([P, GROUP], F32)
                    nc.vector.tensor_copy(xt, xt_ps)
                    flush_pending()
                    pending = (xt, g0)
                flush_pending()
                # last stripe streams out in 512 KiB chunks right behind the
                # drains so the kernel tail is one small chunk, not 4 MiB
                if s == n_stripes - 1:
                    q = 2 * GROUP
                    for c in range(COLS // q):
                        nc.scalar.dma_start(
                            out=y[s * P : (s + 1) * P, c * q : (c + 1) * q],
                            in_=ys[:, c * q : (c + 1) * q],
                        )
                else:
                    nc.scalar.dma_start(out=y[s * P : (s + 1) * P, :], in_=ys)
    return _legalize_waits(nc)


def kernel(x, H, trace=False):
    global _cached_nc
    if _cached_nc is None:
        _cached_nc = _build()
    nc = _cached_nc

    from concourse.bass_utils import run_bass_kernel_spmd

    if trace:
        # register the axon NTFF hook (the agent image lacks the boot-time
        # registration; inject the registry module and replicate the hook
        # from trn_agent_boot)
        import types

        if "antenv.axon_hooks" not in sys.modules:
            mod = types.ModuleType("antenv.axon_hooks")
            mod._hook = None
            mod.set_axon_ntff_profile_hook = lambda h: setattr(mod, "_hook", h)
            mod.get_axon_ntff_profile_hook = lambda: mod._hook
            sys.modules["antenv.axon_hooks"] = mod
            import antenv

            antenv.axon_hooks = mod
        hooks = sys.modules["antenv.axon_hooks"]
        if hooks.get_axon_ntff_profile_hook() is None:
            if "/root/.axon_site" not in sys.path:
                sys.path.insert(0, "/root/.axon_site")
            from trn_agent_boot.trn_boot import _ntff_profile_via_ctypes

            hooks.set_axon_ntff_profile_hook(
                _ntff_profile_via_ctypes("/opt/axon/libaxon_pjrt.so")
            )

    x = np.asarray(x)
    H = np.asarray(H)
    in_maps = [
        {"x": x[i * SHARD : (i + 1) * SHARD], "H": H} for i in range(N_CORES)
    ]
    res = run_bass_kernel_spmd(
        nc, in_maps, core_ids=list(range(N_CORES)), trace=trace
    )
    out = np.concatenate([r["y"] for r in res.results], axis=0)
    if trace:
        kernel.last_exec_time_ns = res.exec_time_ns
        kernel.last_trace = res.instructions_and_trace
    return out
